# revision 54
# baseline (speedup 1.0000x reference)
"""CompressAttn Trainium2 Bass kernel (v2: transposed PV + host normalize).

Problem: compressed-block attention.
  B=2, N=4096, QH=32, KH=2, D=VD=128, KSZ=32, STRIDE=16, M=255 blocks.
  kc[b,m,h,:] = sum_i w_k[i] * (k[b,16m+i,h,:] + pe_k[i,:])   (same for v)
  out = softmax(q @ kc^T * D^-0.5, causal-banded mask) @ vc, zero for n < 31.

Sharding: 8 cores = (batch b in {0,1}) x (query-head quarter hq in {0..3}).
Each core handles 8 query heads that share a single KV head (g = hq//2), so
K/V compression is done once per core.  No collectives needed; host gathers.

Device pipeline per core (all attention matmuls bf16, psum f32):
  1. Compression via banded matmul (bf16): per 128-row chunk c of k
     (stationary) stream [128,16] block-diag weight tile -> psum [d,(t,a)];
     kcT[d,m] = P0[m] + P1[m+1] + bias_k -> bf16.  v likewise -> vcT, then
     PE-transpose to natural vc0/vc1 [m, d] (PV stationaries).
  2. Per (head, 512-col block b): sT[m-chunk, 512] = kcT_chunk^T @ qT (1-2
     matmuls), exp on ScalarE (scale fused), multiplicative staircase mask
     on GpSimd (bf16, one aligned 31-row piece + a 15-col single-row piece).
  3. Transposed PV: oT[vd, 512] = vc_chunk^T(stationary) @ eT(moving),
     1-2 matmuls accumulated in psum.  Denominator row dn[1, 512] via a
     ones[mc,1] stationary matmul over the same eT moving.
  4. oT psum -> sbuf bf16 on DVE (2-4KB/partition DMA packets), dn -> f32.
     Softmax division happens on the host (o / max(dn, eps)); queries n<31
     have dn == 0 and o == 0 exactly.
"""

import ml_dtypes
import numpy as np

import concourse.bacc as bacc
import concourse.mybir as mybir
import concourse.tile as tile
from concourse.bass_utils import run_bass_kernel_spmd

# Problem geometry (hardcoded per contest rules).
B, N, QH, KH, D, VD = 2, 4096, 32, 2, 128, 128
KSZ, STRIDE = 32, 16
M = (N - KSZ) // STRIDE + 1          # 255 compressed blocks
HPC = QH // 4                         # 8 query heads per core
NBLK = N // 512                       # 8 query blocks of 512
SM = float(D) ** -0.5

F32 = mybir.dt.float32
BF16 = mybir.dt.bfloat16


def build_program():
    nc = bacc.Bacc("TRN2", target_bir_lowering=False, debug=False)

    # All inputs are host-pre-arranged so every DMA is contiguous per
    # partition (few, large descriptors — sync-queue descgen is expensive).
    qT_d = nc.dram_tensor("qT", [HPC, D, N], BF16, kind="ExternalInput")
    k_d = nc.dram_tensor("kk", [128, 32 * 128], BF16, kind="ExternalInput")
    v_d = nc.dram_tensor("vv", [128, 32 * 128], BF16, kind="ExternalInput")
    # bf16 blob cols: w01k[0:16] | w01v[16:32] | m01r[32:48] | ident[48:176]
    blob_d = nc.dram_tensor("blob", [128, 176], BF16, kind="ExternalInput")
    bias_d = nc.dram_tensor("biaskv", [128, 2], F32, kind="ExternalInput")
    m01_d = nc.dram_tensor("m01", [128, 8 * 512], BF16, kind="ExternalInput")
    o_d = nc.dram_tensor(
        "o", [HPC, NBLK // 4, VD, 2048], BF16, kind="ExternalOutput"
    )
    dn_d = nc.dram_tensor("dn", [HPC, 2, 4 * 512], F32, kind="ExternalOutput")

    with tile.TileContext(nc) as tc:
        with tc.tile_pool(name="consts", bufs=1) as cp:
            blob = cp.tile([128, 176], BF16)
            biaskv = cp.tile([128, 2], F32)
            m01 = cp.tile([128, 8 * 512], BF16)
            onesc = cp.tile([128, 1], BF16)
            kcT = cp.tile([128, M], BF16)         # [d, m] QK stationary
            vcT = cp.tile([128, 256], BF16)       # [d, t] staging
            vc0 = cp.tile([128, 128], BF16)       # [m 0:128,   d]
            vc1 = cp.tile([128, 128], BF16)       # [m 128:255, d]
            w01k = blob[:, 0:16]
            w01v = blob[:, 16:32]
            m01r = blob[:, 32:48]      # row p%32==31 -> 0, else 1
            ident = blob[:, 48:176]
            biask = biaskv[:, 0:1]
            biasv = biaskv[:, 1:2]

            # ---- attention (+ compression interleaved into the prologue) --
            # Software-pipelined: stageA(i) = QK+exp+mask, stageB(i) =
            # dn+PV+copies+DMA.  stageB(i) is emitted after stageA(i+DEPTH)
            # so the in-order PE queue never waits on scalar/vector work of
            # the same iteration.
            with (
                tc.tile_pool(name="kvload", bufs=1) as kvp,
                tc.tile_pool(name="qp", bufs=2) as qp,
                tc.tile_pool(name="ep", bufs=10) as ep,
                tc.tile_pool(name="op", bufs=3) as op,
                tc.tile_pool(name="dnp", bufs=3) as dnp,
                tc.tile_pool(name="sps", bufs=5, space="PSUM") as sps,
                tc.tile_pool(name="pvs", bufs=2, space="PSUM") as pvs,
                tc.tile_pool(name="dns", bufs=1, space="PSUM") as dns,
            ):
                iters = [(h, b) for h in range(HPC) for b in range(NBLK)]
                st = {}
                qtiles = {}

                def load_q(h):
                    # two half-loads so the head's first blocks can start
                    # before the whole 1MB tile lands
                    qtiles[h] = qp.tile([128, N], BF16, tag="qTh", name="qTh")
                    nc.sync.dma_start(
                        qtiles[h][:, 0 : N // 2], qT_d.ap()[h, :, 0 : N // 2]
                    )
                    nc.sync.dma_start(
                        qtiles[h][:, N // 2 : N], qT_d.ap()[h, :, N // 2 : N]
                    )

                # blob (w01 weights) first — the PE needs it immediately;
                # then k/v (compression gates everything), qT(0), m01 (only
                # needed once exp(0) lands).
                ktile = kvp.tile([128, 32 * 128], BF16)
                vtile = kvp.tile([128, 32 * 128], BF16)
                nc.sync.dma_start(blob[:, :], blob_d.ap())
                nc.sync.dma_start(biaskv[:, :], bias_d.ap())
                nc.sync.dma_start(ktile[:, :], k_d.ap())
                nc.sync.dma_start(vtile[:, :], v_d.ap())
                load_q(0)
                nc.sync.dma_start(m01[:, :], m01_d.ap())
                nc.vector.memset(onesc[:, :], 1.0)

                # p-state warmup: the PE idles ~8us waiting for the k DMA;
                # spin dummy matmuls so the clock ramps to full (3us of
                # continuous execution) before compression starts.
                scratch = kvp.tile([128, 512], BF16)
                nc.vector.memset(scratch[:, :], 0.0)
                wps = pvs.tile([128, 512], F32, tag="pv", name="wps")
                for _ in range(20):
                    nc.tensor.matmul(
                        wps[0:1, :], scratch[:, 0:1], scratch[:, :],
                        start=True, stop=True,
                    )

                # ---- compression ----
                # free layout (t, a): pkT[d, 2t+a] = P_a[t].  Prologue psum
                # comes from the sps pool (same shape/tag) so everything
                # fits the 8 psum banks; the transposes reuse pkT/pvT's
                # banks via tag cycling, with a bf16 bitcast view.
                pkT = sps.tile([128, 512], F32, tag="sT", name="pkT")
                pvT = sps.tile([128, 512], F32, tag="sT", name="pvT")
                for c in range(32):
                    nc.tensor.matmul(
                        pkT[:, 16 * c : 16 * c + 16],
                        ktile[:, 128 * c : 128 * (c + 1)],
                        w01k[:, :],
                        start=True, stop=True,
                    )
                # kcT[d,m] = P0[m] + P1[m+1] + bias_k[d]; these DVE adds run
                # while the PE does the v matmuls below.
                pk3 = pkT[:, :].rearrange("p (t a) -> p t a", a=2)
                nc.vector.tensor_scalar_add(kcT[:, 0:M], pk3[:, 0:M, 0], biask[:, 0:1])
                nc.vector.tensor_add(kcT[:, 0:M], kcT[:, 0:M], pk3[:, 1 : M + 1, 1])
                for c in range(32):
                    nc.tensor.matmul(
                        pvT[:, 16 * c : 16 * c + 16],
                        vtile[:, 128 * c : 128 * (c + 1)],
                        w01v[:, :],
                        start=True, stop=True,
                    )
                pv3 = pvT[:, :].rearrange("p (t a) -> p t a", a=2)
                nc.vector.tensor_scalar_add(vcT[:, 0:M], pv3[:, 0:M, 0], biasv[:, 0:1])
                nc.vector.tensor_add(vcT[:, 0:M], vcT[:, 0:M], pv3[:, 1 : M + 1, 1])
                nc.vector.memset(vcT[:, M : M + 1], 0.0)

                def finish_compression():
                    # transpose vcT -> natural vc [m, d] (PV-T stationaries);
                    # emitted after the first two stageA's so QK(0) doesn't
                    # wait behind the transposes' vcT dependency.
                    tpA = sps.tile([128, 512], F32, tag="sT", name="tpA")
                    tpB = sps.tile([128, 512], F32, tag="sT", name="tpB")
                    tpAv = tpA.bitcast(BF16)[:, 0:128]
                    tpBv = tpB.bitcast(BF16)[:, 0:128]
                    nc.tensor.transpose(tpAv, vcT[:, 0:128], ident[:, :])
                    nc.tensor.transpose(tpBv, vcT[:, 128:256], ident[:, :])
                    nc.vector.tensor_copy(vc0[:, :], tpAv)
                    nc.vector.tensor_copy(vc1[:, :], tpBv)

                def stageA(i):
                    h, b = iters[i]
                    if b == 0 and h + 1 < HPC:
                        load_q(h + 1)       # prefetch next head
                    mr = min(32 * b + 31, M)      # visible m count
                    c0r = min(mr, 128)
                    c1r = mr - 128
                    qs = qtiles[h][:, 512 * b : 512 * (b + 1)]

                    sT0 = sps.tile([128, 512], F32, tag="sT")
                    nc.tensor.matmul(
                        sT0[0:c0r, :], kcT[:, 0:c0r], qs,
                        start=True, stop=True,
                    )
                    eT0 = ep.tile([128, 512], BF16, tag="eT")
                    nc.scalar.activation(
                        eT0[0:c0r, :], sT0[0:c0r, :],
                        mybir.ActivationFunctionType.Exp, scale=SM,
                    )
                    eT1 = None
                    if c1r > 0:
                        sT1 = sps.tile([128, 512], F32, tag="sT")
                        nc.tensor.matmul(
                            sT1[0:c1r, :], kcT[:, 128 : 128 + c1r], qs,
                            start=True, stop=True,
                        )
                        eT1 = ep.tile([128, 512], BF16, tag="eT")
                        nc.scalar.activation(
                            eT1[0:c1r, :], sT1[0:c1r, :],
                            mybir.ActivationFunctionType.Exp, scale=SM,
                        )
    # multiplicative staircase mask: only rows [32b, 32b+31) are
                    # partially masked (one aligned 31-row DVE mul with the
                    # matching m01 variant rows); row 32b-1 just zeroes its
                    # first 15 cols (GpSimd memset).  All other computed rows
                    # are fully visible.
                    # column-split across DVE + GpSimd: both halves run in
                    # parallel right after exp, halving mask-ready latency
                    # on the PE's dn/PV dependency chain.
                    ps, pe_ = 32 * b, 32 * b + 31
                    if pe_ <= 128:            # piece lives in chunk0
                        mj = m01[:, 512 * b : 512 * (b + 1)]
                        et, s1, e1 = eT0, ps, pe_
                    else:                     # chunk1 (b >= 4)
                        mj = m01[:, 512 * (b - 4) : 512 * (b - 3)]
                        et, s1, e1 = eT1, ps - 128, pe_ - 128
                    nc.vector.tensor_mul(
                        et[s1:e1, 0:256], et[s1:e1, 0:256], mj[s1:e1, 0:256]
                    )
                    nc.gpsimd.tensor_mul(
                        et[s1:e1, 256:512], et[s1:e1, 256:512],
                        mj[s1:e1, 256:512]
                    )
                    # row 32b-1 masks cols 0:15; it is the last row of the
                    # aligned window [32b-32, 32b), so multiply that window's
                    # first 15 cols by m01r (zero only in rows p%32==31).
                    pr = 32 * b - 1
                    if 0 <= pr < 128:
                        ws = pr - 31
                        nc.gpsimd.tensor_mul(
                            eT0[ws : ws + 32, 0:15], eT0[ws : ws + 32, 0:15],
                            m01r[ws : ws + 32, 0:15],
                        )
                    elif pr >= 128:
                        ws = pr - 159
                        nc.gpsimd.tensor_mul(
                            eT1[ws : ws + 32, 0:15], eT1[ws : ws + 32, 0:15],
                            m01r[ws : ws + 32, 0:15],
                        )
                    st[i] = (c0r, c1r, eT0, eT1)

                def stageB(i):
                    h, b = iters[i]
                    c0r, c1r, eT0, eT1 = st.pop(i)

                    # transposed PV: oT[vd,512] = vc^T @ eT
                    pvt = pvs.tile([128, 512], F32, tag="pv")
                    nc.tensor.matmul(
                        pvt[:, :], vc0[0:c0r, :], eT0[0:c0r, :],
                        start=True, stop=(c1r <= 0),
                    )
                    if c1r > 0:
                        nc.tensor.matmul(
                            pvt[:, :], vc1[0:c1r, :], eT1[0:c1r, :],
                            start=False, stop=True,
                        )

                    # denominator row dn[1,512] = sum_m eT[m,:]; 2
                    # consecutive blocks share one psum bank at partition
                    # bases 0/64 (matmul psum out base must be 0/32/64) so
                    # one DVE copy serves 2 blocks.
                    j = b % 2
                    if j == 0:
                        st[("dn", h)] = dns.tile(
                            [128, 512], F32, tag="dn", name="dnt"
                        )
                    dnt = st[("dn", h)]
                    drow = dnt[64 * j : 64 * j + 1, :]
                    nc.tensor.matmul(
                        drow, onesc[0:c0r, 0:1], eT0[0:c0r, :],
                        start=True, stop=(c1r <= 0),
                    )
                    if c1r > 0:
                        nc.tensor.matmul(
                            drow, onesc[0:c1r, 0:1], eT1[0:c1r, :],
                            start=False, stop=True,
                        )

                    # psum -> sbuf; 4 consecutive blocks share one [128,2048]
                    # sbuf tile and a single 4KB-per-partition DMA.  dn rows
                    # accumulate into a per-head [65, 2048] tile (psum row 0
                    # -> sbuf row 0, row 64 -> row 64), DMA'd twice per head.
                    jo = b % 4
                    if jo == 0:
                        st[("o", h, b // 4)] = op.tile(
                            [128, 2048], BF16, tag="o", name="o_blk"
                        )
                    o_blk = st[("o", h, b // 4)]
                    nc.vector.tensor_copy(
                        o_blk[:, 512 * jo : 512 * (jo + 1)], pvt[:, :]
                    )
                    if h == HPC - 1 and b >= 4:
                        # final group: flush per block for a shorter tail
                        nc.sync.dma_start(
                            o_d.ap()[h, 1, :, 512 * jo : 512 * (jo + 1)],
                            o_blk[:, 512 * jo : 512 * (jo + 1)],
                        )
                    elif jo == 3:
                        nc.sync.dma_start(o_d.ap()[h, b // 4], o_blk[:, :])
                    if ("dnsb", h) not in st:
                        st[("dnsb", h)] = dnp.tile(
                            [65, 2048], F32, tag="dnsb", name="dnsb"
                        )
                    if j == 1:
                        dnsb = st[("dnsb", h)]
                        q4 = b // 2
                        nc.vector.tensor_copy(
                            dnsb[:, 512 * q4 : 512 * (q4 + 1)], dnt[0:65, :]
                        )
                        if b == NBLK - 1:
                            nc.sync.dma_start(dn_d.ap()[h, 0], dnsb[0:1, :])
                            nc.sync.dma_start(dn_d.ap()[h, 1], dnsb[64:65, :])

                DEPTH = 4
                for i in range(len(iters)):
                    stageA(i)
                    if i == 1:
                        finish_compression()
                    if i >= DEPTH:
                        stageB(i - DEPTH)
                for i in range(len(iters) - DEPTH, len(iters)):
                    stageB(i)
    nc.compile()
    return nc


def make_consts(w_k, pe_k, w_v, pe_v):
    """Host-side constant tensors fed to every core."""
    f = np.float32
    bf = ml_dtypes.bfloat16
    w01k = np.zeros((128, 16), f)
    w01v = np.zeros((128, 16), f)
    for r in range(128):
        j = r // 16
        s = r % 16
        for a in range(2):
            # column layout (j, a): col = 2*j + a, matching psum (t, a)
            w01k[r, 2 * j + a] = w_k[16 * a + s]
            w01v[r, 2 * j + a] = w_v[16 * a + s]
    biask = (w_k[:, None] * pe_k).sum(0).astype(f)[:, None]  # [128,1]
    biasv = (w_v[:, None] * pe_v).sum(0).astype(f)[:, None]
    # variant v: row p = stair(p - 32v + 32); stair(r): n' >= 16r - 481
    m01 = np.ones((8, 128, 512), f)
    for vv in range(8):
        for p in range(128):
            r = p - 32 * vv + 32
            if 0 <= r < 64:
                lo = 16 * r - 481
                if lo >= 512:
                    m01[vv, p, :] = 0.0
                else:
                    m01[vv, p, : max(lo, 0)] = 0.0
    ident = np.eye(128, dtype=f)
    m01r = np.ones((128, 16), f)
    m01r[31::32, :] = 0.0
    blob = np.hstack([w01k, w01v, m01r, ident])          # [128, 176]
    return {
        "blob": np.ascontiguousarray(blob).astype(bf),
        "biaskv": np.ascontiguousarray(np.hstack([biask, biasv])),
        "m01": np.ascontiguousarray(m01.transpose(1, 0, 2).reshape(128, -1)
                                    ).astype(bf),
    }


def make_in_map(q, k, v, consts, core):
    b, hq = core // 4, core % 4
    g = hq // 2
    bf = ml_dtypes.bfloat16
    qT = np.ascontiguousarray(
        q[b, :, 8 * hq : 8 * (hq + 1), :].transpose(1, 2, 0)
    ).astype(bf)  # [8, D, N]
    # device ktile layout: partition r holds chunks c=0..31 of d-rows, i.e.
    # kk[r, 128c + d] = k[128c + r, d]
    kk = np.ascontiguousarray(
        k[b, :, g, :].reshape(32, 128, 128).transpose(1, 0, 2).reshape(128, -1)
    ).astype(bf)
    vv = np.ascontiguousarray(
        v[b, :, g, :].reshape(32, 128, 128).transpose(1, 0, 2).reshape(128, -1)
    ).astype(bf)
    return {"qT": qT, "kk": kk, "vv": vv, **consts}


_CACHE = {}


def _compiled():
    if "nc" not in _CACHE:
        _CACHE["nc"] = build_program()
    return _CACHE["nc"]


def kernel(q, k, v, w_k, pe_k, w_v, pe_v, _trace=False, _trace_kwargs=None):
    q = np.asarray(q, np.float32)
    k = np.asarray(k, np.float32)
    v = np.asarray(v, np.float32)
    consts = make_consts(
        np.asarray(w_k, np.float32), np.asarray(pe_k, np.float32),
        np.asarray(w_v, np.float32), np.asarray(pe_v, np.float32),
    )
    nc = _compiled()
    in_maps = [make_in_map(q, k, v, consts, c) for c in range(8)]
    kw = {}
    if _trace:
        kw = {"trace": True, **(_trace_kwargs or {})}
    res = run_bass_kernel_spmd(nc, in_maps, core_ids=list(range(8)), **kw)
    out = np.empty((B, N, QH, VD), np.float32)
    for c in range(8):
        b, hq = c // 4, c % 4
        oT = np.asarray(res.results[c]["o"], np.float32)    # [8,2,128,2048]
        dnr = np.asarray(res.results[c]["dn"], np.float32)  # [8,2,2048]
        o = oT.transpose(0, 1, 3, 2).reshape(HPC, N, VD)    # [h, n, vd]
        # dn[h, par, 512*(bb//2)+n'] -> [h, n]; n = 512*bb + n', par = bb%2
        d = dnr.reshape(HPC, 2, 4, 512).transpose(0, 2, 1, 3).reshape(HPC, N)
        o /= np.maximum(d, 1e-30)[:, :, None]
        out[b, :, 8 * hq : 8 * (hq + 1), :] = o.transpose(1, 0, 2)
    _CACHE["last_result"] = res
    return out


# revision 55
# speedup vs baseline: 1.0411x; 1.0411x over previous
"""CompressAttn Trainium2 Bass kernel (v2: transposed PV + host normalize).

Problem: compressed-block attention.
  B=2, N=4096, QH=32, KH=2, D=VD=128, KSZ=32, STRIDE=16, M=255 blocks.
  kc[b,m,h,:] = sum_i w_k[i] * (k[b,16m+i,h,:] + pe_k[i,:])   (same for v)
  out = softmax(q @ kc^T * D^-0.5, causal-banded mask) @ vc, zero for n < 31.

Sharding: 8 cores = (batch b in {0,1}) x (query-head quarter hq in {0..3}).
Each core handles 8 query heads that share a single KV head (g = hq//2), so
K/V compression is done once per core.  No collectives needed; host gathers.

Device pipeline per core (all attention matmuls bf16, psum f32):
  1. Compression via banded matmul (bf16): per 128-row chunk c of k
     (stationary) stream [128,16] block-diag weight tile -> psum [d,(t,a)];
     kcT[d,m] = P0[m] + P1[m+1] + bias_k -> bf16.  v likewise -> vcT, then
     PE-transpose to natural vc0/vc1 [m, d] (PV stationaries).
  2. Per (head, 512-col block b): sT[m-chunk, 512] = kcT_chunk^T @ qT (1-2
     matmuls), exp on ScalarE (scale fused), multiplicative staircase mask
     on GpSimd (bf16, one aligned 31-row piece + a 15-col single-row piece).
  3. Transposed PV: oT[vd, 512] = vc_chunk^T(stationary) @ eT(moving),
     1-2 matmuls accumulated in psum.  Denominator row dn[1, 512] via a
     ones[mc,1] stationary matmul over the same eT moving.
  4. oT psum -> sbuf bf16 on DVE (2-4KB/partition DMA packets), dn -> f32.
     Softmax division happens on the host (o / max(dn, eps)); queries n<31
     have dn == 0 and o == 0 exactly.
"""

import ml_dtypes
import numpy as np

import concourse.bacc as bacc
import concourse.mybir as mybir
import concourse.tile as tile
from concourse.bass_utils import run_bass_kernel_spmd

# Problem geometry (hardcoded per contest rules).
B, N, QH, KH, D, VD = 2, 4096, 32, 2, 128, 128
KSZ, STRIDE = 32, 16
M = (N - KSZ) // STRIDE + 1          # 255 compressed blocks
HPC = QH // 4                         # 8 query heads per core
NBLK = N // 512                       # 8 query blocks of 512
SM = float(D) ** -0.5

F32 = mybir.dt.float32
BF16 = mybir.dt.bfloat16


def build_program():
    nc = bacc.Bacc("TRN2", target_bir_lowering=False, debug=False)

    # All inputs are host-pre-arranged so every DMA is contiguous per
    # partition (few, large descriptors — sync-queue descgen is expensive).
    qT_d = nc.dram_tensor("qT", [HPC, D, N], BF16, kind="ExternalInput")
    k_d = nc.dram_tensor("kk", [128, 32 * 128], BF16, kind="ExternalInput")
    v_d = nc.dram_tensor("vv", [128, 32 * 128], BF16, kind="ExternalInput")
    # bf16 blob cols: w01k[0:16] | w01v[16:32] | m01r[32:48] | ident[48:176]
    blob_d = nc.dram_tensor("blob", [128, 176], BF16, kind="ExternalInput")
    bias_d = nc.dram_tensor("biaskv", [128, 2], F32, kind="ExternalInput")
    m01_d = nc.dram_tensor("m01", [128, 8 * 512], BF16, kind="ExternalInput")
    o_d = nc.dram_tensor(
        "o", [HPC, NBLK // 4, VD, 2048], BF16, kind="ExternalOutput"
    )
    dn_d = nc.dram_tensor("dn", [HPC, 2, 4 * 512], F32, kind="ExternalOutput")

    with tile.TileContext(nc) as tc:
        with tc.tile_pool(name="consts", bufs=1) as cp:
            blob = cp.tile([128, 176], BF16)
            biaskv = cp.tile([128, 2], F32)
            m01 = cp.tile([128, 8 * 512], BF16)
            onesc = cp.tile([128, 1], BF16)
            kcT = cp.tile([128, M], BF16)         # [d, m] QK stationary
            vcT = cp.tile([128, 256], BF16)       # [d, t] staging
            vc0 = cp.tile([128, 128], BF16)       # [m 0:128,   d]
            vc1 = cp.tile([128, 128], BF16)       # [m 128:255, d]
            w01k = blob[:, 0:16]
            w01v = blob[:, 16:32]
            m01r = blob[:, 32:48]      # row p%32==31 -> 0, else 1
            ident = blob[:, 48:176]
            biask = biaskv[:, 0:1]
            biasv = biaskv[:, 1:2]

            # ---- attention (+ compression interleaved into the prologue) --
            # Software-pipelined: stageA(i) = QK+exp+mask, stageB(i) =
            # dn+PV+copies+DMA.  stageB(i) is emitted after stageA(i+DEPTH)
            # so the in-order PE queue never waits on scalar/vector work of
            # the same iteration.
            with (
                tc.tile_pool(name="kvload", bufs=1) as kvp,
                tc.tile_pool(name="qp", bufs=2) as qp,
                tc.tile_pool(name="ep", bufs=10) as ep,
                tc.tile_pool(name="op", bufs=3) as op,
                tc.tile_pool(name="dnp", bufs=3) as dnp,
                tc.tile_pool(name="sps", bufs=5, space="PSUM") as sps,
                tc.tile_pool(name="pvs", bufs=2, space="PSUM") as pvs,
                tc.tile_pool(name="dns", bufs=1, space="PSUM") as dns,
            ):
                iters = [(h, b) for h in range(HPC) for b in range(NBLK)]
                st = {}
                qtiles = {}

                def load_q(h):
                    # two half-loads so the head's first blocks can start
                    # before the whole 1MB tile lands
                    qtiles[h] = qp.tile([128, N], BF16, tag="qTh", name="qTh")
                    nc.sync.dma_start(
                        qtiles[h][:, 0 : N // 2], qT_d.ap()[h, :, 0 : N // 2]
                    )
                    nc.sync.dma_start(
                        qtiles[h][:, N // 2 : N], qT_d.ap()[h, :, N // 2 : N]
                    )

                # blob (w01 weights) first — the PE needs it immediately;
                # then k/v (compression gates everything), qT(0), m01 (only
                # needed once exp(0) lands).
                ktile = kvp.tile([128, 32 * 128], BF16)
                vtile = kvp.tile([128, 32 * 128], BF16)
                nc.sync.dma_start(blob[:, :], blob_d.ap())
                nc.sync.dma_start(biaskv[:, :], bias_d.ap())
                nc.sync.dma_start(ktile[:, :], k_d.ap())
                nc.sync.dma_start(vtile[:, :], v_d.ap())
                load_q(0)
                nc.sync.dma_start(m01[:, :], m01_d.ap())
                nc.vector.memset(onesc[:, :], 1.0)

                # p-state warmup: the PE idles ~8us waiting for the k DMA;
                # spin dummy matmuls so the clock ramps to full (3us of
                # continuous execution) before compression starts.
                scratch = kvp.tile([128, 512], BF16)
                nc.vector.memset(scratch[:, :], 0.0)
                wps = pvs.tile([128, 512], F32, tag="pv", name="wps")
                for _ in range(20):
                    nc.tensor.matmul(
                        wps[0:1, :], scratch[:, 0:1], scratch[:, :],
                        start=True, stop=True,
                    )

                # ---- compression ----
                # free layout (t, a): pkT[d, 2t+a] = P_a[t].  Prologue psum
                # comes from the sps pool (same shape/tag) so everything
                # fits the 8 psum banks; the transposes reuse pkT/pvT's
                # banks via tag cycling, with a bf16 bitcast view.
                pkT = sps.tile([128, 512], F32, tag="sT", name="pkT")
                pvT = sps.tile([128, 512], F32, tag="sT", name="pvT")
                for c in range(32):
                    nc.tensor.matmul(
                        pkT[:, 16 * c : 16 * c + 16],
                        ktile[:, 128 * c : 128 * (c + 1)],
                        w01k[:, :],
                        start=True, stop=True,
                    )
                # kcT[d,m] = P0[m] + P1[m+1] + bias_k[d]; these DVE adds run
                # while the PE does the v matmuls below.
                pk3 = pkT[:, :].rearrange("p (t a) -> p t a", a=2)
                nc.vector.tensor_scalar_add(kcT[:, 0:M], pk3[:, 0:M, 0], biask[:, 0:1])
                nc.vector.tensor_add(kcT[:, 0:M], kcT[:, 0:M], pk3[:, 1 : M + 1, 1])
                for c in range(32):
                    nc.tensor.matmul(
                        pvT[:, 16 * c : 16 * c + 16],
                        vtile[:, 128 * c : 128 * (c + 1)],
                        w01v[:, :],
                        start=True, stop=True,
                    )
                pv3 = pvT[:, :].rearrange("p (t a) -> p t a", a=2)
                nc.vector.tensor_scalar_add(vcT[:, 0:M], pv3[:, 0:M, 0], biasv[:, 0:1])
                nc.vector.tensor_add(vcT[:, 0:M], vcT[:, 0:M], pv3[:, 1 : M + 1, 1])
                nc.vector.memset(vcT[:, M : M + 1], 0.0)

                def finish_compression():
                    # transpose vcT -> natural vc [m, d] (PV-T stationaries);
                    # emitted after the first two stageA's so QK(0) doesn't
                    # wait behind the transposes' vcT dependency.
                    tpA = sps.tile([128, 512], F32, tag="sT", name="tpA")
                    tpB = sps.tile([128, 512], F32, tag="sT", name="tpB")
                    tpAv = tpA.bitcast(BF16)[:, 0:128]
                    tpBv = tpB.bitcast(BF16)[:, 0:128]
                    nc.tensor.transpose(tpAv, vcT[:, 0:128], ident[:, :])
                    nc.tensor.transpose(tpBv, vcT[:, 128:256], ident[:, :])
                    nc.vector.tensor_copy(vc0[:, :], tpAv)
                    nc.vector.tensor_copy(vc1[:, :], tpBv)

                def stageA(i):
                    h, b = iters[i]
                    if b == 0 and h + 1 < HPC:
                        load_q(h + 1)       # prefetch next head
                    mr = min(32 * b + 31, M)      # visible m count
                    c0r = min(mr, 128)
                    c1r = mr - 128
                    qs = qtiles[h][:, 512 * b : 512 * (b + 1)]

                    sT0 = sps.tile([128, 512], F32, tag="sT")
                    nc.tensor.matmul(
                        sT0[0:c0r, :], kcT[:, 0:c0r], qs,
                        start=True, stop=True,
                    )
                    eT0 = ep.tile([128, 512], BF16, tag="eT")
                    nc.scalar.activation(
                        eT0[0:c0r, :], sT0[0:c0r, :],
                        mybir.ActivationFunctionType.Exp, scale=SM,
                    )
                    eT1 = None
                    if c1r > 0:
                        sT1 = sps.tile([128, 512], F32, tag="sT")
                        nc.tensor.matmul(
                            sT1[0:c1r, :], kcT[:, 128 : 128 + c1r], qs,
                            start=True, stop=True,
                        )
                        eT1 = ep.tile([128, 512], BF16, tag="eT")
                        nc.scalar.activation(
                            eT1[0:c1r, :], sT1[0:c1r, :],
                            mybir.ActivationFunctionType.Exp, scale=SM,
                        )
    # multiplicative staircase mask: only rows [32b, 32b+31) are
                    # partially masked (one aligned 31-row DVE mul with the
                    # matching m01 variant rows); row 32b-1 just zeroes its
                    # first 15 cols (GpSimd memset).  All other computed rows
                    # are fully visible.
                    # column-split across DVE + GpSimd: both halves run in
                    # parallel right after exp, halving mask-ready latency
                    # on the PE's dn/PV dependency chain.
                    ps, pe_ = 32 * b, 32 * b + 31
                    if pe_ <= 128:            # piece lives in chunk0
                        mj = m01[:, 512 * b : 512 * (b + 1)]
                        et, s1, e1 = eT0, ps, pe_
                    else:                     # chunk1 (b >= 4)
                        mj = m01[:, 512 * (b - 4) : 512 * (b - 3)]
                        et, s1, e1 = eT1, ps - 128, pe_ - 128
                    nc.vector.tensor_mul(
                        et[s1:e1, 0:256], et[s1:e1, 0:256], mj[s1:e1, 0:256]
                    )
                    nc.gpsimd.tensor_mul(
                        et[s1:e1, 256:512], et[s1:e1, 256:512],
                        mj[s1:e1, 256:512]
                    )
                    # row 32b-1 masks cols 0:15; it is the last row of the
                    # aligned window [32b-32, 32b), so multiply that window's
                    # first 15 cols by m01r (zero only in rows p%32==31).
                    pr = 32 * b - 1
                    if 0 <= pr < 128:
                        ws = pr - 31
                        nc.gpsimd.tensor_mul(
                            eT0[ws : ws + 32, 0:15], eT0[ws : ws + 32, 0:15],
                            m01r[ws : ws + 32, 0:15],
                        )
                    elif pr >= 128:
                        ws = pr - 159
                        nc.gpsimd.tensor_mul(
                            eT1[ws : ws + 32, 0:15], eT1[ws : ws + 32, 0:15],
                            m01r[ws : ws + 32, 0:15],
                        )
                    st[i] = (c0r, c1r, eT0, eT1)

                def stageB(i):
                    h, b = iters[i]
                    c0r, c1r, eT0, eT1 = st.pop(i)

                    # transposed PV: oT[vd,512] = vc^T @ eT
                    pvt = pvs.tile([128, 512], F32, tag="pv")
                    nc.tensor.matmul(
                        pvt[:, :], vc0[0:c0r, :], eT0[0:c0r, :],
                        start=True, stop=(c1r <= 0),
                    )
                    if c1r > 0:
                        nc.tensor.matmul(
                            pvt[:, :], vc1[0:c1r, :], eT1[0:c1r, :],
                            start=False, stop=True,
                        )

                    # denominator row dn[1,512] = sum_m eT[m,:]; 2
                    # consecutive blocks share one psum bank at partition
                    # bases 0/64 (matmul psum out base must be 0/32/64) so
                    # one DVE copy serves 2 blocks.
                    j = b % 2
                    if j == 0:
                        st[("dn", h)] = dns.tile(
                            [128, 512], F32, tag="dn", name="dnt"
                        )
                    dnt = st[("dn", h)]
                    drow = dnt[64 * j : 64 * j + 1, :]
                    nc.tensor.matmul(
                        drow, onesc[0:c0r, 0:1], eT0[0:c0r, :],
                        start=True, stop=(c1r <= 0),
                    )
                    if c1r > 0:
                        nc.tensor.matmul(
                            drow, onesc[0:c1r, 0:1], eT1[0:c1r, :],
                            start=False, stop=True,
                        )

                    # psum -> sbuf; 4 consecutive blocks share one [128,2048]
                    # sbuf tile and a single 4KB-per-partition DMA.  dn rows
                    # accumulate into a per-head [65, 2048] tile (psum row 0
                    # -> sbuf row 0, row 64 -> row 64), DMA'd twice per head.
                    jo = b % 4
                    if jo == 0:
                        st[("o", h, b // 4)] = op.tile(
                            [128, 2048], BF16, tag="o", name="o_blk"
                        )
                    o_blk = st[("o", h, b // 4)]
                    nc.vector.tensor_copy(
                        o_blk[:, 512 * jo : 512 * (jo + 1)], pvt[:, :]
                    )
                    if h == HPC - 1 and b >= 4:
                        # final group: flush per block for a shorter tail
                        nc.sync.dma_start(
                            o_d.ap()[h, 1, :, 512 * jo : 512 * (jo + 1)],
                            o_blk[:, 512 * jo : 512 * (jo + 1)],
                        )
                    elif jo == 3:
                        nc.sync.dma_start(o_d.ap()[h, b // 4], o_blk[:, :])
                    if ("dnsb", h) not in st:
                        st[("dnsb", h)] = dnp.tile(
                            [65, 2048], F32, tag="dnsb", name="dnsb"
                        )
                    if j == 1:
                        dnsb = st[("dnsb", h)]
                        q4 = b // 2
                        nc.vector.tensor_copy(
                            dnsb[:, 512 * q4 : 512 * (q4 + 1)], dnt[0:65, :]
                        )
                        if b == NBLK - 1:
                            nc.sync.dma_start(dn_d.ap()[h, 0], dnsb[0:1, :])
                            nc.sync.dma_start(dn_d.ap()[h, 1], dnsb[64:65, :])

                DEPTH = 3
                for i in range(len(iters)):
                    stageA(i)
                    if i == 1:
                        finish_compression()
                    if i >= DEPTH:
                        stageB(i - DEPTH)
                for i in range(len(iters) - DEPTH, len(iters)):
                    stageB(i)
    nc.compile()
    return nc


def make_consts(w_k, pe_k, w_v, pe_v):
    """Host-side constant tensors fed to every core."""
    f = np.float32
    bf = ml_dtypes.bfloat16
    w01k = np.zeros((128, 16), f)
    w01v = np.zeros((128, 16), f)
    for r in range(128):
        j = r // 16
        s = r % 16
        for a in range(2):
            # column layout (j, a): col = 2*j + a, matching psum (t, a)
            w01k[r, 2 * j + a] = w_k[16 * a + s]
            w01v[r, 2 * j + a] = w_v[16 * a + s]
    biask = (w_k[:, None] * pe_k).sum(0).astype(f)[:, None]  # [128,1]
    biasv = (w_v[:, None] * pe_v).sum(0).astype(f)[:, None]
    # variant v: row p = stair(p - 32v + 32); stair(r): n' >= 16r - 481
    m01 = np.ones((8, 128, 512), f)
    for vv in range(8):
        for p in range(128):
            r = p - 32 * vv + 32
            if 0 <= r < 64:
                lo = 16 * r - 481
                if lo >= 512:
                    m01[vv, p, :] = 0.0
                else:
                    m01[vv, p, : max(lo, 0)] = 0.0
    ident = np.eye(128, dtype=f)
    m01r = np.ones((128, 16), f)
    m01r[31::32, :] = 0.0
    blob = np.hstack([w01k, w01v, m01r, ident])          # [128, 176]
    return {
        "blob": np.ascontiguousarray(blob).astype(bf),
        "biaskv": np.ascontiguousarray(np.hstack([biask, biasv])),
        "m01": np.ascontiguousarray(m01.transpose(1, 0, 2).reshape(128, -1)
                                    ).astype(bf),
    }


def make_in_map(q, k, v, consts, core):
    b, hq = core // 4, core % 4
    g = hq // 2
    bf = ml_dtypes.bfloat16
    qT = np.ascontiguousarray(
        q[b, :, 8 * hq : 8 * (hq + 1), :].transpose(1, 2, 0)
    ).astype(bf)  # [8, D, N]
    # device ktile layout: partition r holds chunks c=0..31 of d-rows, i.e.
    # kk[r, 128c + d] = k[128c + r, d]
    kk = np.ascontiguousarray(
        k[b, :, g, :].reshape(32, 128, 128).transpose(1, 0, 2).reshape(128, -1)
    ).astype(bf)
    vv = np.ascontiguousarray(
        v[b, :, g, :].reshape(32, 128, 128).transpose(1, 0, 2).reshape(128, -1)
    ).astype(bf)
    return {"qT": qT, "kk": kk, "vv": vv, **consts}


_CACHE = {}


def _compiled():
    if "nc" not in _CACHE:
        _CACHE["nc"] = build_program()
    return _CACHE["nc"]


def kernel(q, k, v, w_k, pe_k, w_v, pe_v, _trace=False, _trace_kwargs=None):
    q = np.asarray(q, np.float32)
    k = np.asarray(k, np.float32)
    v = np.asarray(v, np.float32)
    consts = make_consts(
        np.asarray(w_k, np.float32), np.asarray(pe_k, np.float32),
        np.asarray(w_v, np.float32), np.asarray(pe_v, np.float32),
    )
    nc = _compiled()
    in_maps = [make_in_map(q, k, v, consts, c) for c in range(8)]
    kw = {}
    if _trace:
        kw = {"trace": True, **(_trace_kwargs or {})}
    res = run_bass_kernel_spmd(nc, in_maps, core_ids=list(range(8)), **kw)
    out = np.empty((B, N, QH, VD), np.float32)
    for c in range(8):
        b, hq = c // 4, c % 4
        oT = np.asarray(res.results[c]["o"], np.float32)    # [8,2,128,2048]
        dnr = np.asarray(res.results[c]["dn"], np.float32)  # [8,2,2048]
        o = oT.transpose(0, 1, 3, 2).reshape(HPC, N, VD)    # [h, n, vd]
        # dn[h, par, 512*(bb//2)+n'] -> [h, n]; n = 512*bb + n', par = bb%2
        d = dnr.reshape(HPC, 2, 4, 512).transpose(0, 2, 1, 3).reshape(HPC, N)
        o /= np.maximum(d, 1e-30)[:, :, None]
        out[b, :, 8 * hq : 8 * (hq + 1), :] = o.transpose(1, 0, 2)
    _CACHE["last_result"] = res
    return out
